# revision 50
# baseline (speedup 1.0000x reference)
"""VDP (variance-propagating) attention kernel for Trainium2, 8 NeuronCores.

Sharding: core c -> (batch b = c//2, head-group g = c%2) [8 heads each].
Each core computes LN + its QKV slice + attention for its 8 heads + the
partial out-projection for its 512 inner columns. Host sums the two
head-group partials per batch. No collectives needed.

Layout trick: everything on-device lives transposed as [feature, token]
(activations) / [contraction, out] (weights), prepared host-side, so the
contraction dim is always on partitions and no on-device transposes are
needed anywhere. LayerNorm stats (reduce over features = partitions) are
done with ones-vector matmuls on the PE; softmax denominators come for
free from a ones-augmented column in the V operand of the mu-attention AV
matmul, and are broadcast back across partitions with a K=1 PE matmul.

Perf notes (CoreSim cost model): matmul cost = out_free x cyc(moving
dtype): fp32 4.0, fp32r 1.0 (free>=256), bf16 1.0, fp8e4+DoubleRow 0.5
with twice the contraction per instruction. The whole sigma path
(positive-sum accumulations) runs as fp8 DoubleRow pairs; mu path stays
bf16; LN stats run on raw fp32 data viewed as fp32r. All activations use
one table (Exp/Ln/Square/Copy) - sqrt is exp(-0.5*ln(var+eps)).

Host pre-scaling: mu-weights x sqrt(512) (un-done at PSUM eviction);
sigma-raw weights get +ln(512) so a single Exp yields 512*softplus(x)
(relative error ~ softplus(x)/2 ~ 0.3% at x ~ -5).
"""

import math
import os
import sys

import numpy as np

for _p in ("/opt/trn_rl_repo", "/root/.axon_site/_ro/trn_rl_repo"):
    if os.path.isdir(_p) and _p not in sys.path:
        sys.path.insert(0, _p)

HEADS = 16
DH = 64
SCALE = DH ** -0.5
EPS = 1e-5
B, N, D = 4, 1024, 1024
HPC = 8          # heads per core
RQK = 1024       # q+k rows per core (2 * 8 heads * 64)
RV = 512         # v rows per core
P = 128

SQ512 = math.sqrt(512.0)          # host scale on mu-weights
LN512 = math.log(512.0)           # host shift on sigma-raw weights
WB = 512.0                        # w fp8 boost (w_true max ~0.14 -> ~70)
SQWB = math.sqrt(WB)
IS512 = 1.0 / 512.0
ISQ512 = 1.0 / SQ512
ISWB = 1.0 / WB
CBIAS = 10.0    # fp8 mean-shift for oT_sg/a2o (compensated via colsum matmul)

_NC_CACHE = {}


def _build_nc(tiny_out=False):
    import concourse.bass as bass  # noqa: F401
    import concourse.tile as tile
    from concourse import bacc, mybir

    f32 = mybir.dt.float32
    AF = mybir.ActivationFunctionType
    ALU = mybir.AluOpType

    nc = bacc.Bacc(None, target_bir_lowering=False)

    f32r_ = mybir.dt.float32r
    io = {}
    for name, shape in [
        ("muT", [D, N]), ("sgT", [D, N]), ("gb", [P, 16]),
        ("c12", [2, RQK]), ("c12v", [2, RV]),
        ("wqk_mu", [D, RQK]), ("wqk_sr", [D, RQK]),
        ("wv_mu", [D, RV]), ("wv_sr", [D, RV]),
        ("wo_mu", [RV, D]), ("wo_sr", [RV, D]), ("onesd", [P, P]),
    ]:
        dt_in = f32r_ if name in ("muT", "wqk_mu", "wv_mu", "onesd",
                                  "c12", "c12v") else f32
        io[name] = nc.dram_tensor(name, shape, dt_in, kind="ExternalInput")
    io["invs"] = nc.dram_tensor("invs", [1, N], f32, kind="Internal")
    if tiny_out:
        for name, shape in [("yT_mu", [D, N]), ("yT_sg", [D, N])]:
            io[name] = nc.dram_tensor(name, shape, f32)
        io["done"] = nc.dram_tensor("done", [1, 16], f32, kind="ExternalOutput")
    else:
        for name, shape in [("yT_mu", [D, N]), ("yT_sg", [D, N])]:
            io[name] = nc.dram_tensor(name, shape, f32, kind="ExternalOutput")

    with tile.TileContext(nc) as tc:
        _emit(nc, tc, io, mybir)
        if tiny_out:
            with tc.tile_pool(name="doneP", bufs=1) as dp:
                dt_t = dp.tile([1, 16], f32)
                nc.vector.memset(dt_t, 1.0)
                nc.sync.dma_start(out=io["done"][:], in_=dt_t)
    nc.compile()
    return nc


def _emit(nc, tc, io, mybir):
    from contextlib import ExitStack

    f32 = mybir.dt.float32
    f32r = mybir.dt.float32r
    bf = mybir.dt.bfloat16
    f8 = mybir.dt.float8e4
    AF = mybir.ActivationFunctionType
    ALU = mybir.AluOpType
    DR = mybir.MatmulPerfMode.DoubleRow

    with ExitStack() as tctx:
        stage = tctx.enter_context(tc.tile_pool(name="stage", bufs=1))
        # persistent SBUF staging: no DRAM round trips between phases
        qk_mu_sb = stage.tile([P, 8, N], bf)    # rows: 0-3 q-blocks, 4-7 k-blocks
        qk_sg_sb = stage.tile([P, 8, N], bf)    # scaled x SQWB (q also x SCALE)
        v_mu_sb = stage.tile([P, 8, HPC * 65], bf)   # per tok-blk: 8 heads x (64 v + ones)
        v_sg_sb = stage.tile([P, 8, RV], bf)
        ones3 = stage.tile([P, P], f32r)
        nc.sync.dma_start(out=ones3, in_=io["onesd"][:])
        ones_bf = stage.tile([P, P], bf)
        nc.vector.memset(ones_bf, 1.0)

        # ============ Phase A: LayerNorm + QKV ============
        # gamma is folded into the mu-weights host-side; the mean/beta terms
        # enter each QKV PSUM group as one K=2 rank-1 matmul (c12 x
        # [-mean; std]); the per-token 1/std lands at eviction. The raw muT
        # tiles stay resident and feed the mu GEMMs directly, so wave 1 runs
        # as soon as the DMAs land, overlapped with the sigma-path prep.
        with ExitStack() as actx:
            acts = actx.enter_context(tc.tile_pool(name="acts", bufs=1))
            smallA = actx.enter_context(tc.tile_pool(name="smallA", bufs=1))
            muts_p = actx.enter_context(tc.tile_pool(name="mutsP", bufs=1))
            as_dr = acts.tile([P, 8, 2, N], f8)    # plane0 = a2T, plane1 = sg_nT
            muts = muts_p.tile([P, 8, N], f32r)
            inv_b = acts.tile([P, N], f32)         # inv/sqrt(512), broadcast

            gb_sb = smallA.tile([P, 16], f32)
            nc.sync.dma_start(out=gb_sb, in_=io["gb"][:])
            c12_sb = smallA.tile([2, RQK], f32r)
            nc.sync.dma_start(out=c12_sb, in_=io["c12"][:])
            c12v_sb = smallA.tile([2, RV], f32r)
            nc.sync.dma_start(out=c12v_sb, in_=io["c12v"][:])
            eps1 = smallA.tile([1, 1], f32)
            nc.vector.memset(eps1, EPS)
            bln = smallA.tile([1, 1], f32)
            nc.vector.memset(bln, -0.5 * LN512)
            nstd2 = smallA.tile([2, N], f32r)      # row0 = -mean, row1 = std
            inv_sb = smallA.tile([1, N], f32r)
            invc = smallA.tile([P, 8], f32)
            wqA = actx.enter_context(tc.tile_pool(name="wqA", bufs=2))
            wvA = actx.enter_context(tc.tile_pool(name="wvA", bufs=1))
            wv_mu = wvA.tile([P, 8, 512], f32r)
            nc.sync.dma_start(out=wv_mu, in_=io["wv_mu"][:].rearrange("(j p) r -> p j r", p=P))
            sgts = wvA.tile([P, 8, N], bf)
            for j in range(8):
                nc.gpsimd.dma_start(out=sgts[:, j, :],
                                    in_=io["sgT"][j * P:(j + 1) * P, :])

            with ExitStack() as ctx:
                rowA = ctx.enter_context(tc.tile_pool(name="rowA", bufs=1))
                onebuf = ctx.enter_context(tc.tile_pool(name="onebuf", bufs=2))
                bcast1 = ctx.enter_context(tc.tile_pool(name="bcast1", bufs=1))
                inv2_b = bcast1.tile([P, N], f32, tag="inv2b")
                nmean_b = bcast1.tile([P, N], f32, tag="nmeanb")
                psS = ctx.enter_context(tc.tile_pool(name="psS", bufs=1, space="PSUM"))
                psA = ctx.enter_context(tc.tile_pool(name="psA", bufs=2, space="PSUM"))

                sum_ps = [psS.tile([1, 512], f32, tag=f"sum{c}", name=f"sum{c}") for c in range(2)]
                sq_ps = [psS.tile([1, 512], f32, tag=f"sq{c}", name=f"sq{c}") for c in range(2)]
                for j in range(8):
                    nc.sync.dma_start(out=muts[:, j, :], in_=io["muT"][j * P:(j + 1) * P, :])
                    mu2 = onebuf.tile([P, N], f32r, tag="mu2")
                    nc.vector.tensor_mul(mu2, muts[:, j, :], muts[:, j, :])
                    for c in range(2):
                        cs = slice(c * 512, (c + 1) * 512)
                        nc.tensor.matmul(sum_ps[c], ones3[:, 0:1], muts[:, j, cs],
                                         start=(j == 0), stop=(j == 7), skip_group_check=True)
                        nc.tensor.matmul(sq_ps[c], ones3[:, 0:1], mu2[:, cs],
                                         start=(j == 0), stop=(j == 7), skip_group_check=True)

                for c in range(2):
                    cs = slice(c * 512, (c + 1) * 512)
                    mean_t = rowA.tile([1, 512], f32, tag="mean")
                    nc.vector.tensor_scalar_mul(mean_t, sum_ps[c], 1.0 / D)
                    nc.vector.tensor_scalar_mul(nstd2[0:1, cs], sum_ps[c], -1.0 / D)
                    m2_t = rowA.tile([1, 512], f32, tag="m2")
                    nc.vector.tensor_mul(m2_t, mean_t, mean_t)
                    var_t = rowA.tile([1, 512], f32, tag="var")
                    nc.vector.scalar_tensor_tensor(var_t, sq_ps[c], 1.0 / D, m2_t,
                                                   ALU.mult, ALU.subtract)
                    lv_t = rowA.tile([1, 512], f32, tag="lv")
                    nc.scalar.activation(lv_t, var_t, AF.Ln, bias=eps1)
                    nc.scalar.activation(inv_sb[:, cs], lv_t, AF.Exp, scale=-0.5, bias=bln)
                    std_t = rowA.tile([1, 512], f32r, tag="stdt", name=f"stdt{c}")
                    nc.scalar.activation(std_t, lv_t, AF.Exp, scale=0.5)
                    nc.gpsimd.dma_start(out=nstd2[1:2, cs], in_=std_t)

                nc.gpsimd.dma_start(out=io["invs"][:], in_=inv_sb)
                nc.gpsimd.dma_start(out=invc,
                                    in_=io["invs"][:].rearrange("o (t p) -> p (o t)", p=P))

                for c in range(2):
                    cs = slice(c * 512, (c + 1) * 512)
                    bp1 = psA.tile([P, 512], f32, tag="bcast")
                    nc.tensor.matmul(bp1, ones3[0:1, :], inv_sb[:, cs], start=True, stop=True)
                    nc.scalar.copy(inv_b[:, cs], bp1)
                    bp2 = psA.tile([P, 512], f32, tag="bcast")
                    nc.tensor.matmul(bp2, ones3[0:1, :], nstd2[0:1, cs], start=True, stop=True)
                    nc.vector.tensor_copy(nmean_b[:, cs], bp2)
                nc.vector.tensor_mul(inv2_b, inv_b, inv_b)

                for j in range(8):
                    d_t = onebuf.tile([P, N], bf, tag="d")
                    nc.gpsimd.tensor_add(d_t, muts[:, j, :], nmean_b)
                    zz = onebuf.tile([P, N], bf, tag="zz")
                    nc.scalar.activation(zz, d_t, AF.Square, scale=gb_sb[:, j:j + 1])
                    s1 = onebuf.tile([P, N], bf, tag="s1")
                    nc.gpsimd.tensor_add(s1, sgts[:, j, :], zz)
                    nc.gpsimd.tensor_mul(as_dr[:, j, 0, :], s1, inv2_b)
                    nc.vector.tensor_mul(as_dr[:, j, 1, :], sgts[:, j, :], inv2_b)

            # --- A2 wave 1: ALL mu GEMMs (ready as soon as mu_nT streams in).
            # fp8 weight planes are staged resident so the sigma wave (which
            # depends on the late as_dr tiles) never blocks mu work on the
            # in-order PE queue.
            with ExitStack() as ctx:
                wq = wqA
                wqd = ctx.enter_context(tc.tile_pool(name="wqd", bufs=1))
                wv = ctx.enter_context(tc.tile_pool(name="wv", bufs=1))
                psQ = ctx.enter_context(tc.tile_pool(name="psQ", bufs=2, space="PSUM"))
                wq_dr = wqd.tile([P, 8, 8, 2, P], f8)      # [*, rb, j, plane, r]
                wv_dr = wv.tile([P, 8, 2, 512], f8)
                nc.gpsimd.tensor_mul(wv_dr[:, :, 1, :], wv_mu, wv_mu)
                nc.vector.memset(v_mu_sb, 1.0)
                for rb in range(8):
                    rsl = slice(rb * P, (rb + 1) * P)
                    wmu = wq.tile([P, 8, P], f32r, tag="wmu")
                    nc.sync.dma_start(out=wmu, in_=io["wqk_mu"][:, rsl].rearrange("(j p) r -> p j r", p=P))
                    nc.gpsimd.tensor_mul(wq_dr[:, rb, :, 1, :], wmu, wmu)
                    for c in range(2):
                        cs = slice(c * 512, (c + 1) * 512)
                        ps_mu = psQ.tile([P, 512], f32, tag="qkmu")
                        for j in range(8):
                            nc.tensor.matmul(ps_mu, wmu[:, j, :], muts[:, j, cs],
                                             start=(j == 0), stop=False)
                        nc.tensor.matmul(ps_mu, c12_sb[:, rsl], nstd2[:, cs],
                                         start=False, stop=True)
                        nc.vector.tensor_mul(qk_mu_sb[:, rb, cs], ps_mu, inv_b[:, cs])
                for tb in range(8):
                    tsl = slice(tb * P, (tb + 1) * P)
                    ps_mu = psQ.tile([P, 512], f32, tag="qkmu")
                    for j in range(8):
                        nc.tensor.matmul(ps_mu, muts[:, j, tsl], wv_mu[:, j, :],
                                         start=(j == 0), stop=False)
                    nc.tensor.matmul(ps_mu, nstd2[:, tsl], c12v_sb,
                                     start=False, stop=True)
                    nc.vector.tensor_scalar_mul(
                        v_mu_sb[:, tb, :].rearrange("p (h c) -> p h c", c=65)[:, :, 0:64],
                        ps_mu.rearrange("p (h c) -> p h c", c=64), invc[:, tb:tb + 1])

                # --- A2 wave 2: ALL sigma GEMMs (fp8 DoubleRow) ---
                for rb in range(8):
                    rsl = slice(rb * P, (rb + 1) * P)
                    wsr = wq.tile([P, 8, P], f32, tag="wsr")
                    nc.sync.dma_start(out=wsr, in_=io["wqk_sr"][:, rsl].rearrange("(j p) r -> p j r", p=P))
                    nc.scalar.activation(wq_dr[:, rb, :, 0, :], wsr, AF.Exp)
                    for c in range(2):
                        cs = slice(c * 512, (c + 1) * 512)
                        ps_sg = psQ.tile([P, 512], f32, tag="qksg")
                        for j in range(8):
                            nc.tensor.matmul(ps_sg, wq_dr[:, rb, j, :, :], as_dr[:, j, :, cs],
                                             start=(j == 0), stop=(j == 7), perf_mode=DR)
                        sgev = (SCALE if rb < 4 else 1.0) * SQWB * IS512
                        nc.scalar.activation(qk_sg_sb[:, rb, cs], ps_sg, AF.Copy, scale=sgev)
                for jh in range(4):
                    wv_sr = wq.tile([P, 2, 512], f32, tag="wsr", name=f"wv_sr{jh}")
                    nc.sync.dma_start(
                        out=wv_sr,
                        in_=io["wv_sr"][jh * 2 * P:(jh + 1) * 2 * P, :].rearrange(
                            "(j p) r -> p j r", p=P))
                    nc.scalar.activation(wv_dr[:, 2 * jh:2 * jh + 2, 0, :], wv_sr, AF.Exp)
                for tb in range(8):
                    tsl = slice(tb * P, (tb + 1) * P)
                    ps_sg = psQ.tile([P, 512], f32, tag="qksg")
                    for j in range(8):
                        nc.tensor.matmul(ps_sg, as_dr[:, j, :, tsl], wv_dr[:, j, :, :],
                                         start=(j == 0), stop=(j == 7), perf_mode=DR)
                    nc.scalar.activation(v_sg_sb[:, tb, :], ps_sg,
                                         AF.Copy, scale=IS512)

        # ============ Phase B + C scope ============
        bc_scope = tctx.enter_context(tc.tile_pool(name="bcs", bufs=1))
        oT_mu_sb = bc_scope.tile([P, 4, N], bf)
        co_dr = bc_scope.tile([P, 4, 2, N], f8)    # plane0 = a2o, plane1 = oT_sg

        # ============ Phase B: attention (all operands already in SBUF) ============
        with ExitStack() as ctx:
            ep = ctx.enter_context(tc.tile_pool(name="ep", bufs=18))
            sb3 = ctx.enter_context(tc.tile_pool(name="sb3", bufs=4))
            ptu = ctx.enter_context(tc.tile_pool(name="ptu", bufs=2))
            outsb = ctx.enter_context(tc.tile_pool(name="outsb", bufs=4))
            smallB = ctx.enter_context(tc.tile_pool(name="smallB", bufs=4))
            wpool = ctx.enter_context(tc.tile_pool(name="wpool", bufs=3))
            psD = ctx.enter_context(tc.tile_pool(name="psD", bufs=1, space="PSUM"))
            psS2 = ctx.enter_context(tc.tile_pool(name="psS2", bufs=1, space="PSUM"))
            psAVm = ctx.enter_context(tc.tile_pool(name="psAVm", bufs=2, space="PSUM"))
            psAVs = ctx.enter_context(tc.tile_pool(name="psAVs", bufs=1, space="PSUM"))
            psDB = ctx.enter_context(tc.tile_pool(name="psDB", bufs=1, space="PSUM"))
            oinB = ctx.enter_context(tc.tile_pool(name="oinB", bufs=2))



            def pass1(hq, c):
                pr, hh = divmod(hq, 2)
                pb = (hq % 2) * 64
                qrb, krb = hq // 2, 4 + hq // 2
                vco = pr * 130 + hh * 65
                cs = slice(c * 512, (c + 1) * 512)
                av_mu = psAVm.tile([65, 512], f32, tag="avmu", name=f"avmu{hq}_{c}")
                e_ts = []
                for kp in range(4):
                    e2 = ep.tile([P, 2, 512], bf, tag="e", name=f"e{hq}_{c}_{kp}")
                    e_ts.append(e2)
                    dots2 = psD.tile([P, 2, 512], f32, tag="dots", name=f"dots{hq}_{c}_{kp}")
                    for i in range(2):
                        kb = 2 * kp + i
                        nc.tensor.matmul(dots2[:, i, :],
                                         qk_mu_sb[pb:pb + 64, krb, kb * P:(kb + 1) * P],
                                         qk_mu_sb[pb:pb + 64, qrb, cs],
                                         start=True, stop=True, skip_group_check=True)
                    nc.scalar.activation(e2, dots2, AF.Exp, scale=SCALE)
                    for i in range(2):
                        kb = 2 * kp + i
                        nc.tensor.matmul(av_mu, v_mu_sb[:, kb, vco:vco + 65], e2[:, i, :],
                                         start=(kb == 0), stop=(kb == 7))
                r_sb = smallB.tile([P, 512], bf, tag="r", name=f"r{hq}_{c}")
                with nc.allow_low_precision(reason="softmax denom fits bf16"):
                    nc.vector.reciprocal(r_sb[64:65, :], av_mu[64:65, :])
                dbp = psDB.tile([P, 512], f32, tag="db", name=f"dbp{hq}_{c}")
                nc.tensor.matmul(dbp, ones_bf[64:65, :], r_sb[64:65, :],
                                 start=True, stop=True)
                db = sb3.tile([P, 512], bf, tag="db_sb", name=f"db{hq}_{c}")
                nc.scalar.copy(db, dbp)
                muo = outsb.tile([64, 512], bf, tag="muo", name=f"muo{hq}_{c}")
                nc.vector.tensor_mul(muo, av_mu[0:64, :], db[0:64, :])
                nc.sync.dma_start(out=oT_mu_sb[pb:pb + 64, qrb, cs], in_=muo)
                return (hq, c, e_ts, db)

            def pass2(stateA, stateB):
                # both heads of a pair: sigma-AV DoubleRow matmuls col-packed
                # via tile_position (0,0)/(0,64) and kb-paired fp8 planes.
                hqA, c, e_tsA, dbA = stateA
                hqB, _, e_tsB, dbB = stateB
                pr = hqA // 2
                qrb, krb = pr, 4 + pr
                cs = slice(c * 512, (c + 1) * 512)
                av2 = psAVs.tile([P, 512], f32, tag="avsg", name=f"avsg{hqA}_{c}")
                for kp in range(4):
                    for hq, pb, e_ts, db in ((hqA, 0, e_tsA, dbA), (hqB, 64, e_tsB, dbB)):
                        sd2 = psS2.tile([P, 2, 512], f32, tag="sd2", name=f"sd{hq}_{c}_{kp}")
                        for i in range(2):
                            kb = 2 * kp + i
                            nc.tensor.matmul(sd2[:, i, :],
                                             qk_sg_sb[pb:pb + 64, krb, kb * P:(kb + 1) * P],
                                             qk_sg_sb[pb:pb + 64, qrb, cs],
                                             start=True, stop=True, skip_group_check=True)
                        sde = ptu.tile([P, 2, 512], bf, tag="sde", name=f"sde{hq}_{c}_{kp}")
                        nc.scalar.copy(sde, sd2)
                        p2 = ptu.tile([P, 2, 512], bf, tag="p", name=f"p{hq}_{c}_{kp}")
                        nc.vector.tensor_mul(
                            p2, e_ts[kp],
                            db[:, :].rearrange("p (o f) -> p o f", o=1)
                            .to_broadcast((P, 2, 512)))
                        q2 = ptu.tile([P, 2, 512], bf, tag="q", name=f"q{hq}_{c}_{kp}")
                        nc.gpsimd.tensor_mul(q2, p2, p2)
                        t2 = ptu.tile([P, 2, 512], bf, tag="t", name=f"t{hq}_{c}_{kp}")
                        nc.gpsimd.tensor_sub(t2, p2, q2)
                        u2 = ptu.tile([P, 2, 512], bf, tag="u", name=f"u{hq}_{c}_{kp}")
                        nc.vector.tensor_mul(u2, t2, t2)
                        w2 = wpool.tile([P, 2, 512], bf, tag="w", name=f"w{hq}_{c}_{kp}")
                        nc.vector.tensor_mul(w2, u2, sde)
                        for i in range(2):
                            kb = 2 * kp + i
                            nc.tensor.matmul(av2[pb:pb + 64, :],
                                             v_sg_sb[:, kb, hq * 64:(hq + 1) * 64],
                                             w2[:, i, :],
                                             start=(kb == 0), stop=(kb == 7),
                                             tile_position=(0, pb),
                                             skip_group_check=True)
                nc.scalar.activation(co_dr[:, qrb, 1, cs], av2, AF.Copy,
                                     scale=ISWB, bias=-CBIAS)
                if c == 1:
                    zsq = oinB.tile([P, N], bf, tag="zsq", name=f"zsq{qrb}")
                    nc.scalar.activation(zsq, oT_mu_sb[:, qrb, :], AF.Square)
                    nc.gpsimd.tensor_add(co_dr[:, qrb, 0, :], co_dr[:, qrb, 1, :], zsq)

            prev = None
            for pr in range(4):
                for c in range(2):
                    curA = pass1(2 * pr, c)
                    curB = pass1(2 * pr + 1, c)
                    if prev is not None:
                        pass2(*prev)
                    prev = (curA, curB)
            pass2(*prev)

        # ============ Phase C: out-projection ============
        with ExitStack() as ctx:
            wo = ctx.enter_context(tc.tile_pool(name="wo", bufs=1))
            evC = ctx.enter_context(tc.tile_pool(name="evC", bufs=4))
            psC = ctx.enter_context(tc.tile_pool(name="psC", bufs=2, space="PSUM"))

            wo_mu = wo.tile([P, 4, D], bf)
            nc.gpsimd.dma_start(out=wo_mu, in_=io["wo_mu"][:].rearrange("(j p) o -> p j o", p=P))
            wo_sr = wo.tile([P, 4, D], f32)
            nc.sync.dma_start(out=wo_sr, in_=io["wo_sr"][:].rearrange("(j p) o -> p j o", p=P))
            wo_dr = wo.tile([P, 4, 2, D], f8)
            nc.scalar.activation(wo_dr[:, :, 0, :], wo_sr, AF.Exp)
            nc.gpsimd.tensor_mul(wo_dr[:, :, 1, :], wo_mu, wo_mu)
            ones_c = wo.tile([P, 2, 1], f8)
            nc.vector.memset(ones_c, 1.0)

            for ob in range(8):
                osl = slice(ob * P, (ob + 1) * P)
                colsum = psC.tile([P, 1], f32, tag="colsum", name=f"cols{ob}")
                for j in range(4):
                    nc.tensor.matmul(colsum, wo_dr[:, j, :, osl], ones_c,
                                     start=(j == 0), stop=(j == 3), perf_mode=DR)
                bias_col = evC.tile([P, 1], f32, tag="bias_col", name=f"bcol{ob}")
                nc.vector.tensor_scalar_mul(bias_col, colsum, CBIAS * IS512)
                for c in range(2):
                    cs = slice(c * 512, (c + 1) * 512)
                    ps_mu = psC.tile([P, 512], f32, tag="ymu")
                    for j in range(4):
                        nc.tensor.matmul(ps_mu, wo_mu[:, j, osl], oT_mu_sb[:, j, cs],
                                         start=(j == 0), stop=(j == 3))
                    ev1 = evC.tile([P, 512], f32, tag="ev1")
                    nc.vector.tensor_scalar_mul(ev1, ps_mu, ISQ512)
                    nc.gpsimd.dma_start(out=io["yT_mu"][osl, cs], in_=ev1)
                    ps_sg = psC.tile([P, 512], f32, tag="ysg")
                    for j in range(4):
                        nc.tensor.matmul(ps_sg, wo_dr[:, j, :, osl], co_dr[:, j, :, cs],
                                         start=(j == 0), stop=(j == 3), perf_mode=DR)
                    ev2 = evC.tile([P, 512], f32, tag="ev2")
                    if c == 0:
                        nc.scalar.activation(ev2, ps_sg, AF.Identity, scale=IS512,
                                             bias=bias_col)
                    else:
                        nc.vector.tensor_scalar(ev2, ps_sg, IS512, bias_col,
                                                ALU.mult, ALU.add)
                    nc.scalar.dma_start(out=io["yT_sg"][osl, cs], in_=ev2)


def _get_nc():
    if "nc" not in _NC_CACHE:
        _NC_CACHE["nc"] = _build_nc()
    return _NC_CACHE["nc"]


def _sp512(x):
    # device computes Exp(this) -> exactly 512*softplus(x)
    x64 = np.asarray(x, np.float64)
    return np.asarray(np.log(512.0 * np.log1p(np.exp(x64))), np.float32)


def _prep_core_inputs(c, mu, sigma, ln_gamma, ln_beta, Wqkv_mu, Wqkv_sigma_raw,
                      Wout_mu, Wout_sigma_raw):
    f = np.float32
    asc = np.ascontiguousarray
    b, g = divmod(c, 2)
    qs = slice(512 * g, 512 * (g + 1))
    ks = slice(1024 + 512 * g, 1024 + 512 * (g + 1))
    vs = slice(2048 + 512 * g, 2048 + 512 * (g + 1))
    gamma = np.asarray(ln_gamma, np.float64)
    beta = np.asarray(ln_beta, np.float64)
    gb = np.zeros((P, 16), f)
    gb[:, :8] = (gamma * SQ512).astype(f).reshape(8, P).T
    gb[:, 8:] = np.asarray(ln_beta, f).reshape(8, P).T
    wqk_mu = np.concatenate([Wqkv_mu[qs], Wqkv_mu[ks]], 0).astype(np.float64)
    wqk_sr = np.concatenate([Wqkv_sigma_raw[qs], Wqkv_sigma_raw[ks]], 0)
    wv_mu64 = np.asarray(Wqkv_mu[vs], np.float64)
    # gamma folded into mu-weights; mean/beta terms folded into a K=2 rank-1
    # correction (rows: c1 = colsum of scaled weights -> * -mean;
    #             c2 = beta-weighted colsum * SQ512 -> * std).
    wqk_g = (wqk_mu * gamma[None, :]) * SQ512
    wv_g = (wv_mu64 * gamma[None, :]).T * SQ512
    c12 = np.zeros((2, RQK), f)
    c12[0] = wqk_g.sum(1).astype(f)
    c12[1] = (wqk_mu @ beta).astype(f) * f(SQ512)
    c12v = np.zeros((2, RV), f)
    c12v[0] = wv_g.sum(0).astype(f)
    c12v[1] = (wv_mu64 @ beta).astype(f) * f(SQ512)
    g2row = (512.0 * gamma * gamma).astype(f)[:, None]
    return {
        "muT": asc(np.asarray(mu[b], f).T),
        "sgT": asc(np.asarray(sigma[b], f).T * g2row),
        "gb": gb,
        "c12": c12,
        "c12v": c12v,
        "wqk_mu": asc(wqk_g.T.astype(f)),
        "wqk_sr": _sp512(asc(np.asarray(wqk_sr, f).T)),
        "wv_mu": asc(wv_g.astype(f)),
        "wv_sr": _sp512(asc(np.asarray(Wqkv_sigma_raw[vs], f).T)),
        "wo_mu": asc(np.asarray(Wout_mu[:, 512 * g:512 * (g + 1)], f).T) * f(SQ512),
        "wo_sr": _sp512(asc(np.asarray(Wout_sigma_raw[:, 512 * g:512 * (g + 1)], f).T)),
        "onesd": np.ones((P, P), f),
    }


def _emulate_core(m):
    """Pure-numpy mirror of the on-device program (for validation only)."""
    import ml_dtypes

    def q8(x):
        return np.asarray(x, np.float32).astype(ml_dtypes.float8_e4m3).astype(np.float32)

    def qb(x):
        return np.asarray(x, np.float32).astype(ml_dtypes.bfloat16).astype(np.float32)

    muT, sg_g = m["muT"], m["sgT"]                   # sgT pre-scaled by 512*gamma^2
    gs = m["gb"][:, :8].T.reshape(-1)[:, None]       # sqrt(512)*gamma, [D,1]
    mean = muT.mean(0, keepdims=True)
    var = muT.var(0, keepdims=True)
    lv = np.log(var + EPS)
    inv_s = np.exp(-0.5 * lv - 0.5 * LN512)          # inv/sqrt(512)
    std = np.exp(0.5 * lv)
    inv2s = inv_s * inv_s                            # inv^2/512
    d = muT - mean
    zz = qb(gs * gs * d * d)
    s1 = qb(sg_g + zz)
    a2T = q8(s1 * inv2s)
    sg_nT = q8(sg_g * inv2s)
    wsig_qk = q8(np.exp(m["wqk_sr"]))        # = 512*softplus exact (host-prescaled)
    wmu2_qk = q8(m["wqk_mu"] ** 2)           # = 512*(gamma*Wmu)^2 (prescaled)
    ps_qk = m["wqk_mu"].T @ muT + np.outer(m["c12"][0], -mean[0]) \
        + np.outer(m["c12"][1], std[0])
    qkT_mu = qb(ps_qk * inv_s)
    qkT_sg = qb((wsig_qk.T @ a2T + wmu2_qk.T @ sg_nT) * IS512 * SQWB)
    qkT_sg[:512] *= SCALE
    wsig_v = q8(np.exp(m["wv_sr"]))
    wmu2_v = q8(m["wv_mu"] ** 2)
    ps_v = muT.T @ m["wv_mu"] + np.outer(-mean[0], m["c12v"][0]) \
        + np.outer(std[0], m["c12v"][1])
    v_mu = qb(ps_v * inv_s.T)
    v_sg = qb((a2T.T @ wsig_v + sg_nT.T @ wmu2_v) * IS512)
    oT_mu = np.zeros((RV, N), np.float32)
    oT_sg = np.zeros((RV, N), np.float32)
    for h in range(HPC):
        hs = slice(h * 64, (h + 1) * 64)
        sT = qkT_mu[512 + h * 64:512 + (h + 1) * 64].T @ qkT_mu[hs]  # [kt, qt]
        e = qb(np.exp(SCALE * sT))
        den = e.sum(0, keepdims=True)
        db = qb(1.0 / den)
        p = qb(e * db)
        oT_mu[hs] = (v_mu[:, hs].T @ e) * db
        sdT = qkT_sg[512 + h * 64:512 + (h + 1) * 64].T @ qkT_sg[hs]
        t = qb((p - 1.0) * p)
        w = qb(qb(t * t) * sdT)
        oT_sg[hs] = (v_sg[:, hs].T @ w) * ISWB
    oT_mu = qb(oT_mu)
    oT_sg8 = q8(oT_sg - CBIAS)
    a2o = q8(oT_sg8 + qb(oT_mu * oT_mu))
    wsig_o = q8(np.exp(m["wo_sr"]))
    wmu2_o = q8(m["wo_mu"] ** 2)
    comp = CBIAS * (wsig_o + wmu2_o).sum(0, keepdims=True).T  # [D,1]
    yT_mu = (m["wo_mu"].T @ oT_mu) * ISQ512
    yT_sg = (wsig_o.T @ a2o + wmu2_o.T @ oT_sg8 + comp) * IS512
    return yT_mu.astype(np.float32), yT_sg.astype(np.float32)


def kernel(mu, sigma, ln_gamma, ln_beta, Wqkv_mu, Wqkv_sigma_raw, Wout_mu,
           Wout_sigma_raw, _trace=False):
    from concourse.bass_utils import run_bass_kernel_spmd

    nc = _get_nc()
    args = (mu, sigma, ln_gamma, ln_beta, Wqkv_mu, Wqkv_sigma_raw, Wout_mu,
            Wout_sigma_raw)
    in_maps = [_prep_core_inputs(c, *args) for c in range(8)]
    res = run_bass_kernel_spmd(nc, in_maps, list(range(8)), trace=_trace)
    out_mu = np.zeros((B, N, D), np.float32)
    out_sg = np.zeros((B, N, D), np.float32)
    for c in range(8):
        b = c // 2
        out_mu[b] += res.results[c]["yT_mu"].T
        out_sg[b] += res.results[c]["yT_sg"].T
    if _trace:
        kernel._last_result = res
    return out_mu, out_sg
